# revision 20
# baseline (speedup 1.0000x reference)
"""Cross-attention kernel for Trainium2, 8-core SPMD.

Problem (hardcoded shapes): B=4, N=4096, S=512, DIM=1024, H=16, D=64.
Sharding: data-parallel over B (4) x tensor-parallel over head-groups (2).
Each core computes 8 heads for one batch; host sums the two head-group
partial projection outputs per batch.

Per-core math (g = head group, b = batch):
  QT = qw_g.T @ x_b.T          [512, 4096]   (q-features on partitions)
  KT = kw_g.T @ ctx_b.T        [512, 512]
  V  = ctx_b @ vw_g            [512, 512]    (s on partitions)
  per head h (64 features), per n-chunk:
    S.T  = KT_h.T-slice @ QT_h [s=512, n]    scores transposed
    E    = exp(S.T * 0.125)                  no mask bias needed: masked V rows
                                             are zeroed and the denominator
                                             column of V holds the 0/1 mask
    O'   = [V_h*m | m].T @ E   [65, n]       row 64 = masked softmax denominator
    O.T  = O'[0:64] * (1/O'[64]) broadcast
  out_partial = O.T-as-lhsT @ pw_g + (host adds proj bias + partner partial)

v3 changes (vs the 302us v2):
  - Head-PAIR score matmuls: heads 2w (partitions 0-63) and 2w+1 (64-127)
    have K=64 contractions on complementary partition halves; emitting the
    two heads' score matmuls back-to-back with separate PSUM banks makes
    them concurrent PE row-group tiles (tile_position auto-derives from
    base_partition) -- ~2x on the scores phase.
  - One [P,2,n] scores PSUM tile per st holding BOTH heads' planes, one
    exp instruction covering both: the banks release together, so the
    pair's matmuls actually align (v4 released them via two serial exps
    1.15us apart and only 31/128 pair events overlapped).
  - Output staging + output dram tensor in fp16: halves output DMA
    (16MB -> 8MB per core); host upcasts and sums partials in fp32.
  - PE warm-up: ~10 dummy matmuls on scratch SBUF at t=0 so the HAM clock
    gate reaches 8/8 (2.4 GHz) before the first real matmul (v2 ran the
    first 26us at 1.2 GHz).
  (v3 also tried fine-grained startup DMAs and small tail chunks; both
  REGRESSED: 256B-partition-line DMAs halved DMA throughput and starved
  the PE for 35us at startup, and 128-row chunks reintroduced exp-latency
  bubbles. Keep v2's whole-tile DMA granularity and uniform 512 chunks.)
v2 scheduling notes (kept):
  - QT(c+1) projection and out-proj(c-1) blocks are emitted as fillers
    inside the attention head loop so the PE has independent work while
    the serial scalar exps drain.
  - Normalization is recip(PSUM denom row -> [1,n] sbuf) on vector,
    gpsimd partition_broadcast, one vector mult.
All matmul operands are fp16 (1 col/cycle on the PE); accumulation fp32.
"""
import os
import numpy as np

P = 128
B, N, S, DIM = 4, 4096, 512, 1024
HEADS, D = 16, 64
HG = 8               # heads per core
GF = HG * D          # 512 features per head-group
CHUNKS = [512] * 7 + [256, 256]   # n-chunk sizes, sum = 4096
KT_TILES = DIM // P  # 8 contraction tiles for projections
SCALE = D ** -0.5

LAST_RESULTS = None
_CACHED_NC = None


def _build():
    import concourse.mybir as mybir
    import concourse.tile as tile
    from concourse import bacc

    f32 = mybir.dt.float32
    f16 = mybir.dt.float16

    nc = bacc.Bacc("TRN2", target_bir_lowering=False, debug=False)

    xT = nc.dram_tensor("xT", [DIM, N], f16, kind="ExternalInput")
    ctxT = nc.dram_tensor("ctxT", [DIM, S], f16, kind="ExternalInput")
    qw = nc.dram_tensor("qw", [DIM, GF], f16, kind="ExternalInput")
    kw = nc.dram_tensor("kw", [DIM, GF], f16, kind="ExternalInput")
    vw = nc.dram_tensor("vw", [DIM, GF], f16, kind="ExternalInput")
    pw = nc.dram_tensor("pw", [GF, DIM], f16, kind="ExternalInput")
    qb = nc.dram_tensor("qb", [P, GF // P], f32, kind="ExternalInput")
    kb = nc.dram_tensor("kb", [P, GF // P], f32, kind="ExternalInput")
    vbm = nc.dram_tensor("vbm", [P, S // P, GF], f32, kind="ExternalInput")
    m01 = nc.dram_tensor("m01", [P, S // P], f32, kind="ExternalInput")
    o = nc.dram_tensor("o", [N, DIM], f16, kind="ExternalOutput")

    JQ = GF // P        # 4 q-feature tiles
    ST = S // P         # 4 s tiles
    NMAX = 512

    xT_r = xT.rearrange("(ko ki) n -> ki ko n", ki=P)
    qw_r = qw.rearrange("(ko ki) m -> ki ko m", ki=P)
    kw_r = kw.rearrange("(ko ki) m -> ki ko m", ki=P)
    vw_r = vw.rearrange("(ko ki) m -> ki ko m", ki=P)
    pw_r = pw.rearrange("(ko ki) m -> ki ko m", ki=P)
    ctx_r = ctxT.rearrange("(ko ki) s -> ki ko s", ki=P)

    offs = [0]
    for cn in CHUNKS:
        offs.append(offs[-1] + cn)
    NCH = len(CHUNKS)

    with tile.TileContext(nc) as tc:
        with (
            tc.tile_pool(name="const", bufs=1) as cpool,
            tc.tile_pool(name="kv", bufs=1) as kvpool,
            tc.tile_pool(name="qt", bufs=2) as qtpool,
            tc.tile_pool(name="xq", bufs=2) as xqpool,
            tc.tile_pool(name="e", bufs=2) as epool,
            tc.tile_pool(name="ot", bufs=2) as otpool,
            tc.tile_pool(name="sm", bufs=2) as smpool,
            tc.tile_pool(name="ost", bufs=2) as ostpool,
            tc.tile_pool(name="ps_p", bufs=2, space="PSUM") as ps_p,
            tc.tile_pool(name="ps_s", bufs=2, space="PSUM") as ps_s,
            tc.tile_pool(name="ps_o", bufs=2, space="PSUM") as ps_o,
        ):
            # ---- PE warm-up: ~4.5us of dummy matmuls with no DMA deps so
            # the HAM clock gate is at 8/8 when the first real matmul issues.
            warm = cpool.tile([P, 640], f16)
            nc.vector.memset(warm[:], 0.0)
            wps = ps_o.tile([P, NMAX], f32, tag="o_ps")
            NWARM = 10
            for i in range(NWARM):
                nc.tensor.matmul(wps[:], warm[:, 0:128], warm[:, 128:640],
                                 start=(i == 0), stop=(i == NWARM - 1))

            # ---- weight / constant tiles ----
            qw_sb = cpool.tile([P, KT_TILES, GF], f16)
            kw_sb = cpool.tile([P, KT_TILES, GF], f16)
            vw_sb = cpool.tile([P, KT_TILES, GF], f16)
            pw_sb = cpool.tile([P, GF // P, DIM], f16)
            ctx_sb = cpool.tile([P, KT_TILES, S], f16)
            qb_sb = cpool.tile([P, JQ], f32)
            kb_sb = cpool.tile([P, JQ], f32)
            vbm_sb = cpool.tile([P, ST, GF], f32)
            m01_sb = cpool.tile([P, ST], f32)

            # startup DMA order: what QT(0, jq=0) needs first (qw jq-slice
            # per k-tile + x chunk0 per k-tile), the rest behind it
            nc.sync.dma_start(qb_sb[:], qb[:])
            for k in range(KT_TILES):
                nc.sync.dma_start(qw_sb[:, k, :], qw_r[:, k, :])

            def emit_qt_dma(c):
                cn = CHUNKS[c]
                xq = xqpool.tile([P, KT_TILES, NMAX], f16, tag="xq")
                for kh in range(2):
                    nc.sync.dma_start(
                        xq[:, 4 * kh:4 * kh + 4, 0:cn],
                        xT_r[:, 4 * kh:4 * kh + 4, offs[c]:offs[c] + cn])
                return xq

            xq0 = emit_qt_dma(0)

            def emit_qt_block(xq, jq, qt, cn, bias_on_vector=False):
                """One QT projection feature-block: 8 matmuls + bias.

                Filler blocks put the bias-add on the vector engine so it
                never sits in the scalar queue between a head pair's exps
                (each 0.7us insertion there delays the pair's AV start).
                """
                ps = ps_p.tile([P, NMAX], f32, tag="proj_ps")
                for k in range(KT_TILES):
                    nc.tensor.matmul(
                        ps[:, 0:cn], qw_sb[:, k, jq * P:(jq + 1) * P],
                        xq[:, k, 0:cn],
                        start=(k == 0), stop=(k == KT_TILES - 1))
                if bias_on_vector:
                    nc.vector.tensor_scalar_add(qt[:, jq, 0:cn], ps[:, 0:cn],
                                                qb_sb[:, jq:jq + 1])
                else:
                    nc.scalar.activation(qt[:, jq, 0:cn], ps[:, 0:cn],
                                         mybir.ActivationFunctionType.Identity,
                                         bias=qb_sb[:, jq:jq + 1])

            # QT for chunk 0 runs while kw/ctx/vw/pw still stream in
            qt0 = qtpool.tile([P, JQ, NMAX], f16, tag="qt")
            for jq in range(JQ):
                emit_qt_block(xq0, jq, qt0, CHUNKS[0])

            # kw/ctx stream on the Activation hwdge queue in parallel with
            # qw/xq0 on the sync queue (scalar is idle at startup).
            nc.scalar.dma_start(kb_sb[:], kb[:])
            for k in range(KT_TILES):
                nc.scalar.dma_start(kw_sb[:, k, :], kw_r[:, k, :])
                nc.scalar.dma_start(ctx_sb[:, k, :], ctx_r[:, k, :])

            # ---- KT = kw.T @ ctxT  -> [128, JQ, S] (kfeat on partitions) ----
            kt_sb = kvpool.tile([P, JQ, S], f16)
            for jk in range(JQ):
                ps = ps_p.tile([P, S], f32, tag="proj_ps")
                for k in range(KT_TILES):
                    nc.tensor.matmul(
                        ps[:], kw_sb[:, k, jk * P:(jk + 1) * P], ctx_sb[:, k, :],
                        start=(k == 0), stop=(k == KT_TILES - 1))
                nc.scalar.activation(kt_sb[:, jk, :], ps[:],
                                     mybir.ActivationFunctionType.Identity,
                                     bias=kb_sb[:, jk:jk + 1])

            for k in range(KT_TILES):
                nc.scalar.dma_start(vw_sb[:, k, :], vw_r[:, k, :])
            nc.scalar.dma_start(vbm_sb[:], vbm[:])
            nc.scalar.dma_start(m01_sb[:], m01[:])

            # ---- V = ctx @ vw -> [128(s), ST, HG, 65] with mask column ----
            # V rows for masked s are zeroed; column D holds the 0/1 mask so
            # masked positions drop out of numerator and denominator alike.
            v_sb = kvpool.tile([P, ST, HG, D + 1], f16)
            for st in range(ST):
                ps = ps_p.tile([P, GF], f32, tag="proj_ps")
                for k in range(KT_TILES):
                    nc.tensor.matmul(
                        ps[:], ctx_sb[:, k, st * P:(st + 1) * P], vw_sb[:, k, :],
                        start=(k == 0), stop=(k == KT_TILES - 1))
                nc.vector.scalar_tensor_tensor(
                    v_sb[:, st, :, 0:D],
                    ps.rearrange("p (h d) -> p h d", h=HG),
                    m01_sb[:, st:st + 1],
                    vbm_sb[:, st, :].rearrange("p (h d) -> p h d", h=HG),
                    mybir.AluOpType.mult, mybir.AluOpType.add)
                nc.scalar.activation(
                    v_sb[:, st, :, D], vbm_sb[:, st, 0:HG],
                    mybir.ActivationFunctionType.Identity,
                    bias=m01_sb[:, st:st + 1], scale=0.0)

            for j in range(JQ):
                nc.sync.dma_start(pw_sb[:, j, :], pw_r[:, j, :])

            # ---- main loop ----
            # Filler blocks (QT(c+1) feature blocks, out-proj(c-1) column
            # blocks) are interleaved between each head-pair's scores and AV
            # matmuls so the PE has independent work while the serial scalar
            # exps for that pair complete.
            qt_cur = qt0
            ot_prev = None       # (ot tile, chunk idx) of chunk c-1
            qt_next = None
            xq_next = None

            for c in range(NCH):
                cn = CHUNKS[c]
                ot = otpool.tile([P, JQ, NMAX], f16, tag="ot")

                fillers = []
                if c + 1 < NCH:
                    xq_next = emit_qt_dma(c + 1)
                    qt_next = qtpool.tile([P, JQ, NMAX], f16, tag="qt")
                    for jq in range(JQ):
                        fillers.append(
                            (emit_qt_block, (xq_next, jq, qt_next,
                                             CHUNKS[c + 1], True)))
                if ot_prev is not None:
                    otp, cprev = ot_prev
                    for ns in range(CHUNKS[cprev] // P):
                        fillers.append(
                            (None, (otp, cprev, ns)))

                def emit_outproj(ot_t, cc, ns):
                    ostage = ostpool.tile([P, DIM], f16, tag="ostage")
                    for fh in range(2):
                        ps = ps_p.tile([P, DIM // 2], f32, tag="proj_ps")
                        for j in range(JQ):
                            nc.tensor.matmul(
                                ps[:],
                                ot_t[:, j, ns * P:(ns + 1) * P],
                                pw_sb[:, j, fh * 512:(fh + 1) * 512],
                                start=(j == 0), stop=(j == JQ - 1))
                        nc.vector.tensor_copy(
                            ostage[:, fh * 512:(fh + 1) * 512], ps[:])
                    nc.sync.dma_start(
                        o[offs[cc] + ns * P: offs[cc] + (ns + 1) * P, :],
                        ostage[:])

                def pop_filler(n):
                    for _ in range(n):
                        if not fillers:
                            return
                        fn, args = fillers.pop(0)
                        if fn is None:
                            emit_outproj(*args)
                        else:
                            fn(*args)

                def norm_pair(ops0, ops1, w):
                    # Both heads' PSUM denominator rows bounce into one SBUF
                    # strip, then ONE reciprocal + ONE 64-partition gpsimd
                    # broadcast serve the pair (the muls only read rows
                    # 0:64, so broadcasting 128 partitions was 2x wasted).
                    # (bounce PSUM first: the custom-DVE reciprocal ucode
                    # misbehaves on PSUM sources)
                    dd = smpool.tile([1, 2 * NMAX], f32, tag="dtmp")
                    nc.vector.tensor_copy(dd[:, 0:cn], ops0[D:D + 1, 0:cn])
                    nc.vector.tensor_copy(dd[:, NMAX:NMAX + cn],
                                          ops1[D:D + 1, 0:cn])
                    rcp = smpool.tile([1, 2 * NMAX], f32, tag="rcp")
                    nc.vector.reciprocal_approx_fast(
                        rcp[:, 0:NMAX + cn], dd[:, 0:NMAX + cn])
                    rb = smpool.tile([D, 2 * NMAX], f32, tag="rb")
                    nc.gpsimd.partition_broadcast(rb[:, 0:NMAX + cn],
                                                  rcp[:, 0:NMAX + cn])
                    nc.vector.tensor_mul(ot[0:64, w, 0:cn],
                                         ops0[0:D, 0:cn], rb[:, 0:cn])
                    nc.vector.tensor_mul(ot[64:128, w, 0:cn],
                                         ops1[0:D, 0:cn],
                                         rb[:, NMAX:NMAX + cn])

                for w in range(HG // 2):
                    # scores for the head pair (2w, 2w+1): K=64 matmuls on
                    # complementary partition halves -> concurrent row-group
                    # tiles when emitted adjacently with separate PSUM banks.
                    # One [P,2,n] PSUM tile per st holds both heads' planes
                    # and ONE exp drains both, so the two banks free at the
                    # same instant and the next st's matmul pair stays
                    # aligned.
                    ee = epool.tile([P, 2, ST, NMAX], f16, tag="e")
                    for st in range(ST):
                        sps = ps_s.tile([P, 2, NMAX], f32, tag="s_ps")
                        nc.tensor.matmul(
                            sps[:, 0, 0:cn],
                            kt_sb[0:64, w, st * P:(st + 1) * P],
                            qt_cur[0:64, w, 0:cn],
                            start=True, stop=True)
                        nc.tensor.matmul(
                            sps[:, 1, 0:cn],
                            kt_sb[64:128, w, st * P:(st + 1) * P],
                            qt_cur[64:128, w, 0:cn],
                            start=True, stop=True)
                        nc.scalar.activation(
                            ee[:, :, st, 0:cn], sps[:, :, 0:cn],
                            mybir.ActivationFunctionType.Exp, scale=SCALE)
                        # independent PE work while the exps drain
                        if st % 2 == 1:
                            pop_filler(1)
                    # AV for both heads (row 64 = denominator)
                    ops0 = ps_o.tile([P, NMAX], f32, tag="o_ps")
                    for st in range(ST):
                        nc.tensor.matmul(
                            ops0[0:D + 1, 0:cn], v_sb[:, st, 2 * w, :],
                            ee[:, 0, st, 0:cn],
                            start=(st == 0), stop=(st == ST - 1))
                    ops1 = ps_o.tile([P, NMAX], f32, tag="o_ps")
                    for st in range(ST):
                        nc.tensor.matmul(
                            ops1[0:D + 1, 0:cn], v_sb[:, st, 2 * w + 1, :],
                            ee[:, 1, st, 0:cn],
                            start=(st == 0), stop=(st == ST - 1))
                    norm_pair(ops0, ops1, w)

                # drain remaining fillers (chunk 0 has only QT blocks;
                # steady state has none left here)
                pop_filler(len(fillers))

                if c + 1 == NCH:
                    # last chunk: out-proj now, nothing left to overlap
                    for ns in range(cn // P):
                        emit_outproj(ot, c, ns)
                else:
                    qt_cur = qt_next
                ot_prev = (ot, c)

    nc.compile()
    return nc


def _get_nc():
    global _CACHED_NC
    if _CACHED_NC is None:
        _CACHED_NC = _build()
    return _CACHED_NC


def kernel(x, context, context_mask, q_w, q_b, kv_w, kv_b, proj_w, proj_b):
    global LAST_RESULTS
    from concourse.bass_utils import run_bass_kernel_spmd

    x = np.asarray(x, dtype=np.float32)
    context = np.asarray(context, dtype=np.float32)
    context_mask = np.asarray(context_mask)
    q_w = np.asarray(q_w, dtype=np.float32)
    q_b = np.asarray(q_b, dtype=np.float32)
    kv_w = np.asarray(kv_w, dtype=np.float32)
    kv_b = np.asarray(kv_b, dtype=np.float32)
    proj_w = np.asarray(proj_w, dtype=np.float32)
    proj_b = np.asarray(proj_b, dtype=np.float32)

    c = np.ascontiguousarray

    in_maps = []
    for dev in range(8):
        b, g = dev // 2, dev % 2
        gs = g * GF
        m01_np = np.where(context_mask[b], np.float32(0.0), np.float32(1.0))
        h16 = np.float16
        in_maps.append({
            "xT": c(x[b].T.astype(h16)),
            "ctxT": c(context[b].T.astype(h16)),
            "qw": c(q_w[:, gs:gs + GF].astype(h16)),
            "kw": c(kv_w[:, gs:gs + GF].astype(h16)),
            "vw": c(kv_w[:, DIM + gs:DIM + gs + GF].astype(h16)),
            "pw": c(proj_w[gs:gs + GF, :].astype(h16)),
            "qb": c(q_b[gs:gs + GF].reshape(GF // P, P).T),
            "kb": c(kv_b[gs:gs + GF].reshape(GF // P, P).T),
            "vbm": c(m01_np.reshape(S // P, P).T[:, :, None]
                     * kv_b[DIM + gs:DIM + gs + GF][None, None, :]).astype(np.float32),
            "m01": c(m01_np.reshape(S // P, P).T),
        })

    nc = _get_nc()
    try:
        res = run_bass_kernel_spmd(nc, in_maps, core_ids=list(range(8)))
    except Exception:
        # transient NRT_EXEC_UNIT_UNRECOVERABLE has been observed once on a
        # wedged core; a straight retry recovers it
        res = run_bass_kernel_spmd(nc, in_maps, core_ids=list(range(8)))
    LAST_RESULTS = res

    out = np.empty((B, N, DIM), dtype=np.float32)
    for b in range(B):
        out[b] = (res.results[2 * b]["o"].astype(np.float32)
                  + res.results[2 * b + 1]["o"].astype(np.float32)
                  + proj_b)
    return out


# revision 28
# speedup vs baseline: 1.0348x; 1.0348x over previous
"""Cross-attention kernel for Trainium2, 8-core SPMD.

Problem (hardcoded shapes): B=4, N=4096, S=512, DIM=1024, H=16, D=64.
Sharding: data-parallel over B (4) x tensor-parallel over head-groups (2).
Each core computes 8 heads for one batch; host sums the two head-group
partial projection outputs per batch.

Per-core math (g = head group, b = batch):
  QT = qw_g.T @ x_b.T          [512, 4096]   (q-features on partitions)
  KT = kw_g.T @ ctx_b.T        [512, 512]
  V  = ctx_b @ vw_g            [512, 512]    (s on partitions)
  per head h (64 features), per n-chunk:
    S.T  = KT_h.T-slice @ QT_h [s=512, n]    scores transposed
    E    = exp(S.T * 0.125)                  no mask bias needed: masked V rows
                                             are zeroed and the denominator
                                             column of V holds the 0/1 mask
    O'   = [V_h*m | m].T @ E   [65, n]       row 64 = masked softmax denominator
    O.T  = O'[0:64] * (1/O'[64]) broadcast
  out_partial = O.T-as-lhsT @ pw_g + (host adds proj bias + partner partial)

v3 changes (vs the 302us v2):
  - Head-PAIR score matmuls: heads 2w (partitions 0-63) and 2w+1 (64-127)
    have K=64 contractions on complementary partition halves; emitting the
    two heads' score matmuls back-to-back with separate PSUM banks makes
    them concurrent PE row-group tiles (tile_position auto-derives from
    base_partition) -- ~2x on the scores phase.
  - One [P,2,n] scores PSUM tile per st holding BOTH heads' planes, one
    exp instruction covering both: the banks release together, so the
    pair's matmuls actually align (v4 released them via two serial exps
    1.15us apart and only 31/128 pair events overlapped).
  - Output staging + output dram tensor in fp16: halves output DMA
    (16MB -> 8MB per core); host upcasts and sums partials in fp32.
  - PE warm-up: ~10 dummy matmuls on scratch SBUF at t=0 so the HAM clock
    gate reaches 8/8 (2.4 GHz) before the first real matmul (v2 ran the
    first 26us at 1.2 GHz).
  (v3 also tried fine-grained startup DMAs and small tail chunks; both
  REGRESSED: 256B-partition-line DMAs halved DMA throughput and starved
  the PE for 35us at startup, and 128-row chunks reintroduced exp-latency
  bubbles. Keep v2's whole-tile DMA granularity and uniform 512 chunks.)
v2 scheduling notes (kept):
  - QT(c+1) projection and out-proj(c-1) blocks are emitted as fillers
    inside the attention head loop so the PE has independent work while
    the serial scalar exps drain.
  - Normalization is recip(PSUM denom row -> [1,n] sbuf) on vector,
    gpsimd partition_broadcast, one vector mult.
All matmul operands are fp16 (1 col/cycle on the PE); accumulation fp32.
"""
import os
import numpy as np

P = 128
B, N, S, DIM = 4, 4096, 512, 1024
HEADS, D = 16, 64
HG = 8               # heads per core
GF = HG * D          # 512 features per head-group
CHUNKS = [512] * 8   # n-chunk sizes, sum = 4096
KT_TILES = DIM // P  # 8 contraction tiles for projections
SCALE = D ** -0.5

LAST_RESULTS = None
_CACHED_NC = None


def _build():
    import concourse.mybir as mybir
    import concourse.tile as tile
    from concourse import bacc

    f32 = mybir.dt.float32
    f16 = mybir.dt.float16

    nc = bacc.Bacc("TRN2", target_bir_lowering=False, debug=False)

    xT = nc.dram_tensor("xT", [DIM, N], f16, kind="ExternalInput")
    ctxT = nc.dram_tensor("ctxT", [DIM, S], f16, kind="ExternalInput")
    qw = nc.dram_tensor("qw", [DIM, GF], f16, kind="ExternalInput")
    kw = nc.dram_tensor("kw", [DIM, GF], f16, kind="ExternalInput")
    vw = nc.dram_tensor("vw", [DIM, GF], f16, kind="ExternalInput")
    pw = nc.dram_tensor("pw", [GF, DIM], f16, kind="ExternalInput")
    qb = nc.dram_tensor("qb", [P, GF // P], f32, kind="ExternalInput")
    kb = nc.dram_tensor("kb", [P, GF // P], f32, kind="ExternalInput")
    vbm = nc.dram_tensor("vbm", [P, S // P, GF], f32, kind="ExternalInput")
    m01 = nc.dram_tensor("m01", [P, S // P], f32, kind="ExternalInput")
    o = nc.dram_tensor("o", [N, DIM], f16, kind="ExternalOutput")

    JQ = GF // P        # 4 q-feature tiles
    ST = S // P         # 4 s tiles
    NMAX = 512

    xT_r = xT.rearrange("(ko ki) n -> ki ko n", ki=P)
    qw_r = qw.rearrange("(ko ki) m -> ki ko m", ki=P)
    kw_r = kw.rearrange("(ko ki) m -> ki ko m", ki=P)
    vw_r = vw.rearrange("(ko ki) m -> ki ko m", ki=P)
    pw_r = pw.rearrange("(ko ki) m -> ki ko m", ki=P)
    ctx_r = ctxT.rearrange("(ko ki) s -> ki ko s", ki=P)

    offs = [0]
    for cn in CHUNKS:
        offs.append(offs[-1] + cn)
    NCH = len(CHUNKS)

    with tile.TileContext(nc) as tc:
        with (
            tc.tile_pool(name="const", bufs=1) as cpool,
            tc.tile_pool(name="kv", bufs=1) as kvpool,
            tc.tile_pool(name="qt", bufs=2) as qtpool,
            tc.tile_pool(name="xq", bufs=2) as xqpool,
            tc.tile_pool(name="e", bufs=2) as epool,
            tc.tile_pool(name="ot", bufs=2) as otpool,
            tc.tile_pool(name="sm", bufs=2) as smpool,
            tc.tile_pool(name="ost", bufs=2) as ostpool,
            tc.tile_pool(name="ps_p", bufs=2, space="PSUM") as ps_p,
            tc.tile_pool(name="ps_s", bufs=2, space="PSUM") as ps_s,
            tc.tile_pool(name="ps_o", bufs=2, space="PSUM") as ps_o,
        ):
            # ---- PE warm-up: ~4.5us of dummy matmuls with no DMA deps so
            # the HAM clock gate is at 8/8 when the first real matmul issues.
            warm = cpool.tile([P, 640], f16)
            nc.vector.memset(warm[:], 0.0)
            wps = ps_o.tile([P, NMAX], f32, tag="o_ps")
            NWARM = 10
            for i in range(NWARM):
                nc.tensor.matmul(wps[:], warm[:, 0:128], warm[:, 128:640],
                                 start=(i == 0), stop=(i == NWARM - 1))

            # ---- weight / constant tiles ----
            qw_sb = cpool.tile([P, KT_TILES, GF], f16)
            kw_sb = cpool.tile([P, KT_TILES, GF], f16)
            vw_sb = cpool.tile([P, KT_TILES, GF], f16)
            pw_sb = cpool.tile([P, GF // P, DIM], f16)
            ctx_sb = cpool.tile([P, KT_TILES, S], f16)
            qb_sb = cpool.tile([P, JQ], f32)
            kb_sb = cpool.tile([P, JQ], f32)
            vbm_sb = cpool.tile([P, ST, GF], f32)
            m01_sb = cpool.tile([P, ST], f32)

            # startup DMA order: what QT(0, jq=0) needs first (qw jq-slice
            # per k-tile + x chunk0 per k-tile), the rest behind it
            nc.sync.dma_start(qb_sb[:], qb[:])
            for k in range(KT_TILES):
                nc.sync.dma_start(qw_sb[:, k, :], qw_r[:, k, :])

            def emit_qt_dma(c):
                cn = CHUNKS[c]
                xq = xqpool.tile([P, KT_TILES, NMAX], f16, tag="xq")
                for kh in range(2):
                    nc.sync.dma_start(
                        xq[:, 4 * kh:4 * kh + 4, 0:cn],
                        xT_r[:, 4 * kh:4 * kh + 4, offs[c]:offs[c] + cn])
                return xq

            xq0 = emit_qt_dma(0)

            def emit_qt_block(xq, jq, qt, cn):
                """One QT projection feature-block: 8 matmuls + bias."""
                ps = ps_p.tile([P, NMAX], f32, tag="proj_ps")
                for k in range(KT_TILES):
                    nc.tensor.matmul(
                        ps[:, 0:cn], qw_sb[:, k, jq * P:(jq + 1) * P],
                        xq[:, k, 0:cn],
                        start=(k == 0), stop=(k == KT_TILES - 1))
                nc.scalar.activation(qt[:, jq, 0:cn], ps[:, 0:cn],
                                     mybir.ActivationFunctionType.Identity,
                                     bias=qb_sb[:, jq:jq + 1])

            # QT for chunk 0 runs while kw/ctx/vw/pw still stream in
            qt0 = qtpool.tile([P, JQ, NMAX], f16, tag="qt")
            for jq in range(JQ):
                emit_qt_block(xq0, jq, qt0, CHUNKS[0])

            nc.sync.dma_start(kb_sb[:], kb[:])
            for k in range(KT_TILES):
                nc.sync.dma_start(kw_sb[:, k, :], kw_r[:, k, :])
                nc.sync.dma_start(ctx_sb[:, k, :], ctx_r[:, k, :])

            # ---- KT = kw.T @ ctxT  -> [128, JQ, S] (kfeat on partitions) ----
            kt_sb = kvpool.tile([P, JQ, S], f16)
            for jk in range(JQ):
                ps = ps_p.tile([P, S], f32, tag="proj_ps")
                for k in range(KT_TILES):
                    nc.tensor.matmul(
                        ps[:], kw_sb[:, k, jk * P:(jk + 1) * P], ctx_sb[:, k, :],
                        start=(k == 0), stop=(k == KT_TILES - 1))
                nc.scalar.activation(kt_sb[:, jk, :], ps[:],
                                     mybir.ActivationFunctionType.Identity,
                                     bias=kb_sb[:, jk:jk + 1])

            for k in range(KT_TILES):
                nc.sync.dma_start(vw_sb[:, k, :], vw_r[:, k, :])
            nc.sync.dma_start(vbm_sb[:], vbm[:])
            nc.sync.dma_start(m01_sb[:], m01[:])

            # ---- V = ctx @ vw -> [128(s), ST, HG, 65] with mask column ----
            # V rows for masked s are zeroed; column D holds the 0/1 mask so
            # masked positions drop out of numerator and denominator alike.
            v_sb = kvpool.tile([P, ST, HG, D + 1], f16)
            for st in range(ST):
                ps = ps_p.tile([P, GF], f32, tag="proj_ps")
                for k in range(KT_TILES):
                    nc.tensor.matmul(
                        ps[:], ctx_sb[:, k, st * P:(st + 1) * P], vw_sb[:, k, :],
                        start=(k == 0), stop=(k == KT_TILES - 1))
                nc.vector.scalar_tensor_tensor(
                    v_sb[:, st, :, 0:D],
                    ps.rearrange("p (h d) -> p h d", h=HG),
                    m01_sb[:, st:st + 1],
                    vbm_sb[:, st, :].rearrange("p (h d) -> p h d", h=HG),
                    mybir.AluOpType.mult, mybir.AluOpType.add)
                nc.scalar.activation(
                    v_sb[:, st, :, D], vbm_sb[:, st, 0:HG],
                    mybir.ActivationFunctionType.Identity,
                    bias=m01_sb[:, st:st + 1], scale=0.0)

            for j in range(JQ):
                nc.sync.dma_start(pw_sb[:, j, :], pw_r[:, j, :])

            # ---- main loop ----
            # Filler blocks (QT(c+1) feature blocks, out-proj(c-1) column
            # blocks) are interleaved between each head-pair's scores and AV
            # matmuls so the PE has independent work while the serial scalar
            # exps for that pair complete.
            # The AV+norm for a pair is SOFTWARE-PIPELINED one pair behind
            # (including across chunk boundaries): pair w's four serial exps
            # complete on scalar while the PE runs pair w-1's AV and filler
            # blocks, so the exp chain never gates the PE.

            def emit_av_pair(ee_t, w, ot_t, cn_t):
                # AV for both heads (row 64 = denominator), then the pair's
                # normalization: both PSUM denominator rows bounce into one
                # SBUF strip, ONE reciprocal + ONE 64-partition gpsimd
                # broadcast serve the pair, two muls write ot.
                ops0 = ps_o.tile([P, NMAX], f32, tag="o_ps")
                for st in range(ST):
                    nc.tensor.matmul(
                        ops0[0:D + 1, 0:cn_t], v_sb[:, st, 2 * w, :],
                        ee_t[:, 0, st, 0:cn_t],
                        start=(st == 0), stop=(st == ST - 1))
                ops1 = ps_o.tile([P, NMAX], f32, tag="o_ps")
                for st in range(ST):
                    nc.tensor.matmul(
                        ops1[0:D + 1, 0:cn_t], v_sb[:, st, 2 * w + 1, :],
                        ee_t[:, 1, st, 0:cn_t],
                        start=(st == 0), stop=(st == ST - 1))
                dd = smpool.tile([1, 2 * NMAX], f32, tag="dtmp")
                nc.vector.tensor_copy(dd[:, 0:cn_t], ops0[D:D + 1, 0:cn_t])
                nc.vector.tensor_copy(dd[:, NMAX:NMAX + cn_t],
                                      ops1[D:D + 1, 0:cn_t])
                rcp = smpool.tile([1, 2 * NMAX], f32, tag="rcp")
                nc.vector.reciprocal_approx_fast(
                    rcp[:, 0:NMAX + cn_t], dd[:, 0:NMAX + cn_t])
                rb = smpool.tile([D, 2 * NMAX], f32, tag="rb")
                nc.gpsimd.partition_broadcast(rb[:, 0:NMAX + cn_t],
                                              rcp[:, 0:NMAX + cn_t])
                nc.vector.tensor_mul(ot_t[0:64, w, 0:cn_t],
                                     ops0[0:D, 0:cn_t], rb[:, 0:cn_t])
                nc.vector.tensor_mul(ot_t[64:128, w, 0:cn_t],
                                     ops1[0:D, 0:cn_t],
                                     rb[:, NMAX:NMAX + cn_t])

            qt_cur = qt0
            ot_prev = None       # (ot tile, chunk idx) of chunk c-1
            qt_next = None
            xq_next = None
            pending = None       # (ee, w, ot, cn) awaiting AV+norm

            for c in range(NCH):
                cn = CHUNKS[c]
                ot = otpool.tile([P, JQ, NMAX], f16, tag="ot")

                fillers = []
                if c + 1 < NCH:
                    xq_next = emit_qt_dma(c + 1)
                    qt_next = qtpool.tile([P, JQ, NMAX], f16, tag="qt")
                    for jq in range(JQ):
                        fillers.append(
                            (emit_qt_block, (xq_next, jq, qt_next,
                                             CHUNKS[c + 1])))
                if ot_prev is not None:
                    otp, cprev = ot_prev
                    for ns in range(CHUNKS[cprev] // P):
                        fillers.append(
                            (None, (otp, cprev, ns)))

                def emit_outproj(ot_t, cc, ns):
                    ostage = ostpool.tile([P, DIM], f16, tag="ostage")
                    for fh in range(2):
                        ps = ps_p.tile([P, DIM // 2], f32, tag="proj_ps")
                        for j in range(JQ):
                            nc.tensor.matmul(
                                ps[:],
                                ot_t[:, j, ns * P:(ns + 1) * P],
                                pw_sb[:, j, fh * 512:(fh + 1) * 512],
                                start=(j == 0), stop=(j == JQ - 1))
                        nc.vector.tensor_copy(
                            ostage[:, fh * 512:(fh + 1) * 512], ps[:])
                    nc.sync.dma_start(
                        o[offs[cc] + ns * P: offs[cc] + (ns + 1) * P, :],
                        ostage[:])

                def pop_filler(n):
                    for _ in range(n):
                        if not fillers:
                            return
                        fn, args = fillers.pop(0)
                        if fn is None:
                            emit_outproj(*args)
                        else:
                            fn(*args)

                for w in range(HG // 2):
                    # scores for the head pair (2w, 2w+1): K=64 matmuls on
                    # complementary partition halves -> concurrent row-group
                    # tiles when emitted adjacently with separate PSUM banks.
                    # One [P,2,n] PSUM tile per st holds both heads' planes
                    # and ONE exp drains both, so the two banks free at the
                    # same instant and the next st's matmul pair stays
                    # aligned.
                    ee = epool.tile([P, 2, ST, NMAX], f16, tag="e")
                    for st in range(ST):
                        sps = ps_s.tile([P, 2, NMAX], f32, tag="s_ps")
                        nc.tensor.matmul(
                            sps[:, 0, 0:cn],
                            kt_sb[0:64, w, st * P:(st + 1) * P],
                            qt_cur[0:64, w, 0:cn],
                            start=True, stop=True)
                        nc.tensor.matmul(
                            sps[:, 1, 0:cn],
                            kt_sb[64:128, w, st * P:(st + 1) * P],
                            qt_cur[64:128, w, 0:cn],
                            start=True, stop=True)
                        nc.scalar.activation(
                            ee[:, :, st, 0:cn], sps[:, :, 0:cn],
                            mybir.ActivationFunctionType.Exp, scale=SCALE)
                        if st == 1:
                            # previous pair's AV+norm runs while this pair's
                            # exps drain (also covers the st2 PSUM-bank wait
                            # on exp(st0)); flushing here, before any pop,
                            # also guarantees norm(3,c-1) precedes any
                            # out-proj(c-1) filler -> no PE-order deadlock
                            if pending is not None:
                                emit_av_pair(*pending)
                                pending = None
                            pop_filler(1)
                    pending = (ee, w, ot, cn)
                    pop_filler(1)

                # drain remaining fillers (chunk 0 has only QT blocks;
                # steady state has none left here)
                pop_filler(len(fillers))

                if c + 1 == NCH:
                    # last chunk: drain the final pair, then out-proj with
                    # nothing left to overlap
                    emit_av_pair(*pending)
                    pending = None
                    for ns in range(cn // P):
                        emit_outproj(ot, c, ns)
                else:
                    qt_cur = qt_next
                ot_prev = (ot, c)

    nc.compile()
    return nc


def _get_nc():
    global _CACHED_NC
    if _CACHED_NC is None:
        _CACHED_NC = _build()
    return _CACHED_NC


def kernel(x, context, context_mask, q_w, q_b, kv_w, kv_b, proj_w, proj_b):
    global LAST_RESULTS
    from concourse.bass_utils import run_bass_kernel_spmd

    x = np.asarray(x, dtype=np.float32)
    context = np.asarray(context, dtype=np.float32)
    context_mask = np.asarray(context_mask)
    q_w = np.asarray(q_w, dtype=np.float32)
    q_b = np.asarray(q_b, dtype=np.float32)
    kv_w = np.asarray(kv_w, dtype=np.float32)
    kv_b = np.asarray(kv_b, dtype=np.float32)
    proj_w = np.asarray(proj_w, dtype=np.float32)
    proj_b = np.asarray(proj_b, dtype=np.float32)

    c = np.ascontiguousarray

    in_maps = []
    for dev in range(8):
        b, g = dev // 2, dev % 2
        gs = g * GF
        m01_np = np.where(context_mask[b], np.float32(0.0), np.float32(1.0))
        h16 = np.float16
        in_maps.append({
            "xT": c(x[b].T.astype(h16)),
            "ctxT": c(context[b].T.astype(h16)),
            "qw": c(q_w[:, gs:gs + GF].astype(h16)),
            "kw": c(kv_w[:, gs:gs + GF].astype(h16)),
            "vw": c(kv_w[:, DIM + gs:DIM + gs + GF].astype(h16)),
            "pw": c(proj_w[gs:gs + GF, :].astype(h16)),
            "qb": c(q_b[gs:gs + GF].reshape(GF // P, P).T),
            "kb": c(kv_b[gs:gs + GF].reshape(GF // P, P).T),
            "vbm": c(m01_np.reshape(S // P, P).T[:, :, None]
                     * kv_b[DIM + gs:DIM + gs + GF][None, None, :]).astype(np.float32),
            "m01": c(m01_np.reshape(S // P, P).T),
        })

    nc = _get_nc()
    try:
        res = run_bass_kernel_spmd(nc, in_maps, core_ids=list(range(8)))
    except Exception:
        # transient NRT_EXEC_UNIT_UNRECOVERABLE has been observed once on a
        # wedged core; a straight retry recovers it
        res = run_bass_kernel_spmd(nc, in_maps, core_ids=list(range(8)))
    LAST_RESULTS = res

    out = np.empty((B, N, DIM), dtype=np.float32)
    for b in range(B):
        out[b] = (res.results[2 * b]["o"].astype(np.float32)
                  + res.results[2 * b + 1]["o"].astype(np.float32)
                  + proj_b)
    return out


# revision 31
# speedup vs baseline: 1.0372x; 1.0023x over previous
"""Cross-attention kernel for Trainium2, 8-core SPMD.

Problem (hardcoded shapes): B=4, N=4096, S=512, DIM=1024, H=16, D=64.
Sharding: data-parallel over B (4) x tensor-parallel over head-groups (2).
Each core computes 8 heads for one batch; host sums the two head-group
partial projection outputs per batch.

Per-core math (g = head group, b = batch):
  QT = qw_g.T @ x_b.T          [512, 4096]   (q-features on partitions)
  KT = kw_g.T @ ctx_b.T        [512, 512]
  V  = ctx_b @ vw_g            [512, 512]    (s on partitions)
  per head h (64 features), per n-chunk:
    S.T  = KT_h.T-slice @ QT_h [s=512, n]    scores transposed
    E    = exp(S.T * 0.125)                  no mask bias needed: masked V rows
                                             are zeroed and the denominator
                                             column of V holds the 0/1 mask
    O'   = [V_h*m | m].T @ E   [65, n]       row 64 = masked softmax denominator
    O.T  = O'[0:64] * (1/O'[64]) broadcast
  out_partial = O.T-as-lhsT @ pw_g + (host adds proj bias + partner partial)

v3 changes (vs the 302us v2):
  - Head-PAIR score matmuls: heads 2w (partitions 0-63) and 2w+1 (64-127)
    have K=64 contractions on complementary partition halves; emitting the
    two heads' score matmuls back-to-back with separate PSUM banks makes
    them concurrent PE row-group tiles (tile_position auto-derives from
    base_partition) -- ~2x on the scores phase.
  - One [P,2,n] scores PSUM tile per st holding BOTH heads' planes, one
    exp instruction covering both: the banks release together, so the
    pair's matmuls actually align (v4 released them via two serial exps
    1.15us apart and only 31/128 pair events overlapped).
  - Output staging + output dram tensor in fp16: halves output DMA
    (16MB -> 8MB per core); host upcasts and sums partials in fp32.
  - PE warm-up: ~10 dummy matmuls on scratch SBUF at t=0 so the HAM clock
    gate reaches 8/8 (2.4 GHz) before the first real matmul (v2 ran the
    first 26us at 1.2 GHz).
  (v3 also tried fine-grained startup DMAs and small tail chunks; both
  REGRESSED: 256B-partition-line DMAs halved DMA throughput and starved
  the PE for 35us at startup, and 128-row chunks reintroduced exp-latency
  bubbles. Keep v2's whole-tile DMA granularity and uniform 512 chunks.)
v2 scheduling notes (kept):
  - QT(c+1) projection and out-proj(c-1) blocks are emitted as fillers
    inside the attention head loop so the PE has independent work while
    the serial scalar exps drain.
  - Normalization is recip(PSUM denom row -> [1,n] sbuf) on vector,
    gpsimd partition_broadcast, one vector mult.
All matmul operands are fp16 (1 col/cycle on the PE); accumulation fp32.
"""
import os
import numpy as np

P = 128
B, N, S, DIM = 4, 4096, 512, 1024
HEADS, D = 16, 64
HG = 8               # heads per core
GF = HG * D          # 512 features per head-group
CHUNKS = [512] * 7 + [256, 256]   # n-chunk sizes, sum = 4096
KT_TILES = DIM // P  # 8 contraction tiles for projections
SCALE = D ** -0.5

LAST_RESULTS = None
_CACHED_NC = None


def _build():
    import concourse.mybir as mybir
    import concourse.tile as tile
    from concourse import bacc

    f32 = mybir.dt.float32
    f16 = mybir.dt.float16

    nc = bacc.Bacc("TRN2", target_bir_lowering=False, debug=False)

    xT = nc.dram_tensor("xT", [DIM, N], f16, kind="ExternalInput")
    ctxT = nc.dram_tensor("ctxT", [DIM, S], f16, kind="ExternalInput")
    qw = nc.dram_tensor("qw", [DIM, GF], f16, kind="ExternalInput")
    kw = nc.dram_tensor("kw", [DIM, GF], f16, kind="ExternalInput")
    vw = nc.dram_tensor("vw", [DIM, GF], f16, kind="ExternalInput")
    pw = nc.dram_tensor("pw", [GF, DIM], f16, kind="ExternalInput")
    qb = nc.dram_tensor("qb", [P, GF // P], f32, kind="ExternalInput")
    kb = nc.dram_tensor("kb", [P, GF // P], f32, kind="ExternalInput")
    vbm = nc.dram_tensor("vbm", [P, S // P, GF], f32, kind="ExternalInput")
    m01 = nc.dram_tensor("m01", [P, S // P], f32, kind="ExternalInput")
    o = nc.dram_tensor("o", [N, DIM], f16, kind="ExternalOutput")

    JQ = GF // P        # 4 q-feature tiles
    ST = S // P         # 4 s tiles
    NMAX = 512

    xT_r = xT.rearrange("(ko ki) n -> ki ko n", ki=P)
    qw_r = qw.rearrange("(ko ki) m -> ki ko m", ki=P)
    kw_r = kw.rearrange("(ko ki) m -> ki ko m", ki=P)
    vw_r = vw.rearrange("(ko ki) m -> ki ko m", ki=P)
    pw_r = pw.rearrange("(ko ki) m -> ki ko m", ki=P)
    ctx_r = ctxT.rearrange("(ko ki) s -> ki ko s", ki=P)

    offs = [0]
    for cn in CHUNKS:
        offs.append(offs[-1] + cn)
    NCH = len(CHUNKS)

    with tile.TileContext(nc) as tc:
        with (
            tc.tile_pool(name="const", bufs=1) as cpool,
            tc.tile_pool(name="kv", bufs=1) as kvpool,
            tc.tile_pool(name="qt", bufs=2) as qtpool,
            tc.tile_pool(name="xq", bufs=2) as xqpool,
            tc.tile_pool(name="e", bufs=2) as epool,
            tc.tile_pool(name="ot", bufs=2) as otpool,
            tc.tile_pool(name="sm", bufs=2) as smpool,
            tc.tile_pool(name="ost", bufs=2) as ostpool,
            tc.tile_pool(name="ps_p", bufs=2, space="PSUM") as ps_p,
            tc.tile_pool(name="ps_s", bufs=2, space="PSUM") as ps_s,
            tc.tile_pool(name="ps_o", bufs=2, space="PSUM") as ps_o,
        ):
            # ---- PE warm-up: ~4.5us of dummy matmuls with no DMA deps so
            # the HAM clock gate is at 8/8 when the first real matmul issues.
            warm = cpool.tile([P, 640], f16)
            nc.vector.memset(warm[:], 0.0)
            wps = ps_o.tile([P, NMAX], f32, tag="o_ps")
            NWARM = 26   # ~8us of dummy matmuls: spans the DMA ramp-up so
            #              the HAM clock gate hits 8/8 by ~4us and the PE
            #              never idles before the first weights land
            for i in range(NWARM):
                nc.tensor.matmul(wps[:], warm[:, 0:128], warm[:, 128:640],
                                 start=(i == 0), stop=(i == NWARM - 1))

            # ---- weight / constant tiles ----
            qw_sb = cpool.tile([P, KT_TILES, GF], f16)
            kw_sb = cpool.tile([P, KT_TILES, GF], f16)
            vw_sb = cpool.tile([P, KT_TILES, GF], f16)
            pw_sb = cpool.tile([P, GF // P, DIM], f16)
            ctx_sb = cpool.tile([P, KT_TILES, S], f16)
            qb_sb = cpool.tile([P, JQ], f32)
            kb_sb = cpool.tile([P, JQ], f32)
            vbm_sb = cpool.tile([P, ST, GF], f32)
            m01_sb = cpool.tile([P, ST], f32)

            # startup DMA order: what QT(0, jq=0) needs first (qw jq-slice
            # per k-tile + x chunk0 per k-tile), the rest behind it
            nc.sync.dma_start(qb_sb[:], qb[:])
            for k in range(KT_TILES):
                nc.sync.dma_start(qw_sb[:, k, :], qw_r[:, k, :])

            def emit_qt_dma(c):
                cn = CHUNKS[c]
                xq = xqpool.tile([P, KT_TILES, NMAX], f16, tag="xq")
                for kh in range(2):
                    nc.sync.dma_start(
                        xq[:, 4 * kh:4 * kh + 4, 0:cn],
                        xT_r[:, 4 * kh:4 * kh + 4, offs[c]:offs[c] + cn])
                return xq

            xq0 = emit_qt_dma(0)

            def emit_qt_block(xq, jq, qt, cn):
                """One QT projection feature-block: 8 matmuls + bias."""
                ps = ps_p.tile([P, NMAX], f32, tag="proj_ps")
                for k in range(KT_TILES):
                    nc.tensor.matmul(
                        ps[:, 0:cn], qw_sb[:, k, jq * P:(jq + 1) * P],
                        xq[:, k, 0:cn],
                        start=(k == 0), stop=(k == KT_TILES - 1))
                nc.scalar.activation(qt[:, jq, 0:cn], ps[:, 0:cn],
                                     mybir.ActivationFunctionType.Identity,
                                     bias=qb_sb[:, jq:jq + 1])

            # QT for chunk 0 runs while kw/ctx/vw/pw still stream in
            qt0 = qtpool.tile([P, JQ, NMAX], f16, tag="qt")
            for jq in range(JQ):
                emit_qt_block(xq0, jq, qt0, CHUNKS[0])

            nc.sync.dma_start(kb_sb[:], kb[:])
            for k in range(KT_TILES):
                nc.sync.dma_start(kw_sb[:, k, :], kw_r[:, k, :])
                nc.sync.dma_start(ctx_sb[:, k, :], ctx_r[:, k, :])

            # ---- KT = kw.T @ ctxT  -> [128, JQ, S] (kfeat on partitions) ----
            kt_sb = kvpool.tile([P, JQ, S], f16)
            for jk in range(JQ):
                ps = ps_p.tile([P, S], f32, tag="proj_ps")
                for k in range(KT_TILES):
                    nc.tensor.matmul(
                        ps[:], kw_sb[:, k, jk * P:(jk + 1) * P], ctx_sb[:, k, :],
                        start=(k == 0), stop=(k == KT_TILES - 1))
                nc.scalar.activation(kt_sb[:, jk, :], ps[:],
                                     mybir.ActivationFunctionType.Identity,
                                     bias=kb_sb[:, jk:jk + 1])

            for k in range(KT_TILES):
                nc.sync.dma_start(vw_sb[:, k, :], vw_r[:, k, :])
            nc.sync.dma_start(vbm_sb[:], vbm[:])
            nc.sync.dma_start(m01_sb[:], m01[:])

            # ---- V = ctx @ vw -> [128(s), ST, HG, 65] with mask column ----
            # V rows for masked s are zeroed; column D holds the 0/1 mask so
            # masked positions drop out of numerator and denominator alike.
            v_sb = kvpool.tile([P, ST, HG, D + 1], f16)
            for st in range(ST):
                ps = ps_p.tile([P, GF], f32, tag="proj_ps")
                for k in range(KT_TILES):
                    nc.tensor.matmul(
                        ps[:], ctx_sb[:, k, st * P:(st + 1) * P], vw_sb[:, k, :],
                        start=(k == 0), stop=(k == KT_TILES - 1))
                nc.vector.scalar_tensor_tensor(
                    v_sb[:, st, :, 0:D],
                    ps.rearrange("p (h d) -> p h d", h=HG),
                    m01_sb[:, st:st + 1],
                    vbm_sb[:, st, :].rearrange("p (h d) -> p h d", h=HG),
                    mybir.AluOpType.mult, mybir.AluOpType.add)
                nc.scalar.activation(
                    v_sb[:, st, :, D], vbm_sb[:, st, 0:HG],
                    mybir.ActivationFunctionType.Identity,
                    bias=m01_sb[:, st:st + 1], scale=0.0)

            for j in range(JQ):
                nc.sync.dma_start(pw_sb[:, j, :], pw_r[:, j, :])

            # ---- main loop ----
            # Filler blocks (QT(c+1) feature blocks, out-proj(c-1) column
            # blocks) are interleaved between each head-pair's scores and AV
            # matmuls so the PE has independent work while the serial scalar
            # exps for that pair complete.
            # The AV+norm for a pair is SOFTWARE-PIPELINED one pair behind
            # (including across chunk boundaries): pair w's four serial exps
            # complete on scalar while the PE runs pair w-1's AV and filler
            # blocks, so the exp chain never gates the PE.

            def emit_av_pair(ee_t, w, ot_t, cn_t):
                # AV for both heads (row 64 = denominator), then the pair's
                # normalization: both PSUM denominator rows bounce into one
                # SBUF strip, ONE reciprocal + ONE 64-partition gpsimd
                # broadcast serve the pair, two muls write ot.
                ops0 = ps_o.tile([P, NMAX], f32, tag="o_ps")
                for st in range(ST):
                    nc.tensor.matmul(
                        ops0[0:D + 1, 0:cn_t], v_sb[:, st, 2 * w, :],
                        ee_t[:, 0, st, 0:cn_t],
                        start=(st == 0), stop=(st == ST - 1))
                ops1 = ps_o.tile([P, NMAX], f32, tag="o_ps")
                for st in range(ST):
                    nc.tensor.matmul(
                        ops1[0:D + 1, 0:cn_t], v_sb[:, st, 2 * w + 1, :],
                        ee_t[:, 1, st, 0:cn_t],
                        start=(st == 0), stop=(st == ST - 1))
                dd = smpool.tile([1, 2 * NMAX], f32, tag="dtmp")
                nc.vector.tensor_copy(dd[:, 0:cn_t], ops0[D:D + 1, 0:cn_t])
                nc.vector.tensor_copy(dd[:, NMAX:NMAX + cn_t],
                                      ops1[D:D + 1, 0:cn_t])
                rcp = smpool.tile([1, 2 * NMAX], f32, tag="rcp")
                nc.vector.reciprocal_approx_fast(
                    rcp[:, 0:NMAX + cn_t], dd[:, 0:NMAX + cn_t])
                rb = smpool.tile([D, 2 * NMAX], f32, tag="rb")
                nc.gpsimd.partition_broadcast(rb[:, 0:NMAX + cn_t],
                                              rcp[:, 0:NMAX + cn_t])
                nc.vector.tensor_mul(ot_t[0:64, w, 0:cn_t],
                                     ops0[0:D, 0:cn_t], rb[:, 0:cn_t])
                nc.vector.tensor_mul(ot_t[64:128, w, 0:cn_t],
                                     ops1[0:D, 0:cn_t],
                                     rb[:, NMAX:NMAX + cn_t])

            qt_cur = qt0
            ot_prev = None       # (ot tile, chunk idx) of chunk c-1
            qt_next = None
            xq_next = None
            pending = None       # (ee, w, ot, cn) awaiting AV+norm

            for c in range(NCH):
                cn = CHUNKS[c]
                ot = otpool.tile([P, JQ, NMAX], f16, tag="ot")

                fillers = []
                if c + 1 < NCH:
                    xq_next = emit_qt_dma(c + 1)
                    qt_next = qtpool.tile([P, JQ, NMAX], f16, tag="qt")
                    for jq in range(JQ):
                        fillers.append(
                            (emit_qt_block, (xq_next, jq, qt_next,
                                             CHUNKS[c + 1])))
                if ot_prev is not None:
                    otp, cprev = ot_prev
                    for ns in range(CHUNKS[cprev] // P):
                        fillers.append(
                            (None, (otp, cprev, ns)))

                def emit_outproj(ot_t, cc, ns):
                    ostage = ostpool.tile([P, DIM], f16, tag="ostage")
                    for fh in range(2):
                        ps = ps_p.tile([P, DIM // 2], f32, tag="proj_ps")
                        for j in range(JQ):
                            nc.tensor.matmul(
                                ps[:],
                                ot_t[:, j, ns * P:(ns + 1) * P],
                                pw_sb[:, j, fh * 512:(fh + 1) * 512],
                                start=(j == 0), stop=(j == JQ - 1))
                        nc.vector.tensor_copy(
                            ostage[:, fh * 512:(fh + 1) * 512], ps[:])
                    nc.sync.dma_start(
                        o[offs[cc] + ns * P: offs[cc] + (ns + 1) * P, :],
                        ostage[:])

                def pop_filler(n):
                    for _ in range(n):
                        if not fillers:
                            return
                        fn, args = fillers.pop(0)
                        if fn is None:
                            emit_outproj(*args)
                        else:
                            fn(*args)

                for w in range(HG // 2):
                    # scores for the head pair (2w, 2w+1): K=64 matmuls on
                    # complementary partition halves -> concurrent row-group
                    # tiles when emitted adjacently with separate PSUM banks.
                    # One [P,2,n] PSUM tile per st holds both heads' planes
                    # and ONE exp drains both, so the two banks free at the
                    # same instant and the next st's matmul pair stays
                    # aligned.
                    ee = epool.tile([P, 2, ST, NMAX], f16, tag="e")
                    for st in range(ST):
                        sps = ps_s.tile([P, 2, NMAX], f32, tag="s_ps")
                        nc.tensor.matmul(
                            sps[:, 0, 0:cn],
                            kt_sb[0:64, w, st * P:(st + 1) * P],
                            qt_cur[0:64, w, 0:cn],
                            start=True, stop=True)
                        nc.tensor.matmul(
                            sps[:, 1, 0:cn],
                            kt_sb[64:128, w, st * P:(st + 1) * P],
                            qt_cur[64:128, w, 0:cn],
                            start=True, stop=True)
                        nc.scalar.activation(
                            ee[:, :, st, 0:cn], sps[:, :, 0:cn],
                            mybir.ActivationFunctionType.Exp, scale=SCALE)
                        if st == 1:
                            # previous pair's AV+norm runs while this pair's
                            # exps drain (also covers the st2 PSUM-bank wait
                            # on exp(st0)); flushing here, before any pop,
                            # also guarantees norm(3,c-1) precedes any
                            # out-proj(c-1) filler -> no PE-order deadlock
                            if pending is not None:
                                emit_av_pair(*pending)
                                pending = None
                            # at a chunk entry whose only fillers are
                            # out-proj(c-1) blocks (no QT: next-to-last and
                            # last chunks), don't pop here -- the block
                            # would stall the PE behind norm(3,c-1)'s
                            # vector chain and let HAM re-throttle
                            if w > 0 or c + 1 < NCH:
                                pop_filler(1)
                    pending = (ee, w, ot, cn)
                    pop_filler(1)

                # drain remaining fillers (chunk 0 has only QT blocks;
                # steady state has none left here)
                pop_filler(len(fillers))

                if c + 1 == NCH:
                    # last chunk: drain the final pair, then out-proj with
                    # nothing left to overlap
                    emit_av_pair(*pending)
                    pending = None
                    for ns in range(cn // P):
                        emit_outproj(ot, c, ns)
                else:
                    qt_cur = qt_next
                ot_prev = (ot, c)

    nc.compile()
    return nc


def _get_nc():
    global _CACHED_NC
    if _CACHED_NC is None:
        _CACHED_NC = _build()
    return _CACHED_NC


def kernel(x, context, context_mask, q_w, q_b, kv_w, kv_b, proj_w, proj_b):
    global LAST_RESULTS
    from concourse.bass_utils import run_bass_kernel_spmd

    x = np.asarray(x, dtype=np.float32)
    context = np.asarray(context, dtype=np.float32)
    context_mask = np.asarray(context_mask)
    q_w = np.asarray(q_w, dtype=np.float32)
    q_b = np.asarray(q_b, dtype=np.float32)
    kv_w = np.asarray(kv_w, dtype=np.float32)
    kv_b = np.asarray(kv_b, dtype=np.float32)
    proj_w = np.asarray(proj_w, dtype=np.float32)
    proj_b = np.asarray(proj_b, dtype=np.float32)

    c = np.ascontiguousarray

    in_maps = []
    for dev in range(8):
        b, g = dev // 2, dev % 2
        gs = g * GF
        m01_np = np.where(context_mask[b], np.float32(0.0), np.float32(1.0))
        h16 = np.float16
        in_maps.append({
            "xT": c(x[b].T.astype(h16)),
            "ctxT": c(context[b].T.astype(h16)),
            "qw": c(q_w[:, gs:gs + GF].astype(h16)),
            "kw": c(kv_w[:, gs:gs + GF].astype(h16)),
            "vw": c(kv_w[:, DIM + gs:DIM + gs + GF].astype(h16)),
            "pw": c(proj_w[gs:gs + GF, :].astype(h16)),
            "qb": c(q_b[gs:gs + GF].reshape(GF // P, P).T),
            "kb": c(kv_b[gs:gs + GF].reshape(GF // P, P).T),
            "vbm": c(m01_np.reshape(S // P, P).T[:, :, None]
                     * kv_b[DIM + gs:DIM + gs + GF][None, None, :]).astype(np.float32),
            "m01": c(m01_np.reshape(S // P, P).T),
        })

    nc = _get_nc()
    try:
        res = run_bass_kernel_spmd(nc, in_maps, core_ids=list(range(8)))
    except Exception:
        # transient NRT_EXEC_UNIT_UNRECOVERABLE has been observed once on a
        # wedged core; a straight retry recovers it
        res = run_bass_kernel_spmd(nc, in_maps, core_ids=list(range(8)))
    LAST_RESULTS = res

    out = np.empty((B, N, DIM), dtype=np.float32)
    for b in range(B):
        out[b] = (res.results[2 * b]["o"].astype(np.float32)
                  + res.results[2 * b + 1]["o"].astype(np.float32)
                  + proj_b)
    return out
